# revision 86
# baseline (speedup 1.0000x reference)
"""Multi-head attention (B=2, N=2048, C=1024, H=16) on 8 trn2 NeuronCores.

Tensor-parallel over heads: core c computes heads {2c, 2c+1} for both batch
elements and emits a partial output y_c = attn_out_c @ W_out[local rows]
(bf16 partials); the host sums the 8 partials and adds b_out.

Per-core pipeline (single TileContext, fully unrolled):
  - x^T loaded once into SBUF (bf16, host pre-transposed so every DMA is a
    contiguous [128, 512] block); weight chunks and window-0 x interleave
    on opposite DMA queues so the first QKV chain starts ASAP.
  - QKV^T projection with stacked per-head weights ([128, 128] stationary).
  - S^T = K @ Q^T as a pair of K=64 matmuls row-tiled onto disjoint row
    groups of the PE array (head h in rows h*64.. via tile_position
    auto-derived from base partitions): the pair streams concurrently, so
    it costs ~one N=512 matmul (~217ns) instead of two.
  - P^T = exp(S^T / 32) on ScalarE straight from PSUM ([128, 1024] ops,
    ~1.11us each; 128 of them = ~142us, the critical engine chain).
  - PV via ones-augmented V (65th stationary column accumulates softmax
    denominators for free). V transposed on the PE (128x128 tiles).
  - Normalization: evict ss+pe FIRST (frees the pvs PSUM banks the next
    q-chunk's PV needs), then reciprocal_approx_fast + GpSimd
    partition_broadcast + DVE multiply -> out^T (bf16).
  - Output projection all-bf16 (fp32 moving operands stream at half rate,
    so bf16 wout/outT halve both the matmul and its LDWEIGHTS).

Scheduling (emission order IS per-engine program order for Tile):
  - Each iteration emits S^T/exp for kc+1 and kc+2 before the PV pair of
    kc: the S stationaries load while the previous pair streams, and
    ScalarE holds a two-deep exp queue (it gates the steady state).
  - QKV work for batch 1 and the deferred projection stores are split
    into ~1us parts injected one-per-iteration into batch-0's attention
    loop; the misc PSUM pool (2 bufs) requires at most one other misc
    allocation between a part that opens a chain accumulator and the part
    that closes it -- the slotting here guarantees that.
  - The deferred-projection queue is shared across the two batches so
    batch-0's tail projections drain inside batch-1's loop.
Never emit a consumer before its producer: reads of not-yet-written SBUF
regions silently bind to stale contents.

Measured: ~220us on-device in the fast clock state (~264us when the part
lands in the throttled P0 state; both states observed for identical
binaries). PE union ~176us, ScalarE ~150us. absmax error ~5.2e-3 of the
output scale vs the fp32 reference (bf16 operands + fp8-free attention:
fp8 Q/K was tried and REJECTED -- softmax washout does not protect the
max-error metric on concentrated-attention queries, rel err hit 2.3e-2).
"""
import os
import sys

sys.path.insert(0, "/opt/trn_rl_repo")

import ml_dtypes
import numpy as np

import concourse.bacc as bacc
import concourse.mybir as mybir
import concourse.tile as tile
from concourse import bass_utils
from concourse.masks import make_identity

F32 = mybir.dt.float32
F32R = mybir.dt.float32r
BF16 = mybir.dt.bfloat16
F8E4 = mybir.dt.float8e4
NPBF16 = ml_dtypes.bfloat16
NPF8E4 = ml_dtypes.float8_e4m3
DR = mybir.MatmulPerfMode.DoubleRow

EMB = 1024
HEADS = 16
B = 2
SEQ = 2048
D = 64
NCORES = 8
HPC = HEADS // NCORES          # heads per core = 2
LD = HPC * D                   # local head dim = 128
TSEQ = B * SEQ                 # 4096
CC = EMB // 128                # contraction chunks = 8
SCALE = float(EMB) ** -0.5     # 1/32

QCH = 512                      # q chunk (free dim of S^T matmuls)
NQ = SEQ // QCH                # 4 q-chunks per batch
NK = SEQ // 128                # 16 k-chunks per batch


def _round_fp32r(x: np.ndarray) -> np.ndarray:
    bits = np.ascontiguousarray(x, dtype=np.float32).view(np.uint32)
    out = ((bits.astype(np.uint64) + 0x800) & 0xFFFFF000).astype(np.uint32)
    return out.view(np.float32)


def _build():
    nc = bacc.Bacc("TRN2", target_bir_lowering=False, debug=False,
                   num_devices=NCORES)

    xT = nc.dram_tensor("xT", [CC, 128, TSEQ], BF16, kind="ExternalInput")
    wqkv = nc.dram_tensor("wqkv", [128, CC * 3 * LD], BF16,
                          kind="ExternalInput")
    bqkv = nc.dram_tensor("bqkv", [LD, 3], F32, kind="ExternalInput")
    wout = nc.dram_tensor("wout", [LD, EMB], BF16, kind="ExternalInput")
    y = nc.dram_tensor("y", [TSEQ // 128, 128, EMB], BF16,
                       kind="ExternalOutput")
    dbg = os.environ.get("KDBG") == "1"
    if dbg:
        d_kt = nc.dram_tensor("d_kt", [LD, SEQ], BF16, kind="ExternalOutput")
        d_qt = nc.dram_tensor("d_qt", [LD, SEQ], BF16, kind="ExternalOutput")
        d_va = nc.dram_tensor("d_va", [NK, 128, 2, 66], BF16,
                              kind="ExternalOutput")
        d_ot = nc.dram_tensor("d_ot", [LD, SEQ], BF16,
                              kind="ExternalOutput")

    xT_c = xT.ap()
    wqkv_c = wqkv.ap()

    with tile.TileContext(nc) as tc:
        with (
            tc.tile_pool(name="persist", bufs=1) as persist,
            tc.tile_pool(name="xt", bufs=2) as xtp,
            tc.tile_pool(name="vt", bufs=2) as vtp,
            tc.tile_pool(name="psb", bufs=6) as psb,
            tc.tile_pool(name="norm", bufs=3) as normp,
            tc.tile_pool(name="yout", bufs=10) as youtp,
            tc.tile_pool(name="ps_st", bufs=2, space="PSUM") as ps_st,
            tc.tile_pool(name="ps_pv", bufs=1, space="PSUM") as ps_pv,
            tc.tile_pool(name="ps_misc", bufs=2, space="PSUM") as ps_misc,
        ):
            # ---- constants / weights (wall+bias first so QKV can start) ----
            ident = persist.tile([128, 128], BF16, tag="ident")
            make_identity(nc, ident[:])
            bqkv_sb = persist.tile([LD, 3], F32, tag="bqkv")
            nc.gpsimd.dma_start(bqkv_sb[:], bqkv.ap())
            bias_sb = {nm: bqkv_sb[:, i:i + 1]
                       for i, nm in enumerate(("q", "k", "v"))}
            wall = persist.tile([128, CC * 3 * LD], BF16, tag="wall")
            wchunk = 3 * LD

            xfull = {}

            def load_x_window(w):
                for kc in range(CC):
                    t = persist.tile([128, 512], BF16, tag=f"xf{kc}_{w}",
                                     name=f"xf{kc}_{w}")
                    eng = nc.gpsimd if kc % 2 else nc.sync
                    eng.dma_start(t[:], xT_c[kc, :, w * 512:(w + 1) * 512])
                    xfull[kc, w] = t

            # interleave weight chunks and window-0 x tiles across THREE
            # queues (scalar is free once its ACT table load issues) so the
            # first QKV chain's full input set lands ASAP
            startq = [nc.sync, nc.gpsimd, nc.scalar]
            for kc in range(CC):
                startq[kc % 3].dma_start(
                    wall[:, kc * wchunk:(kc + 1) * wchunk],
                    wqkv_c[:, kc * wchunk:(kc + 1) * wchunk])
                t = persist.tile([128, 512], BF16, tag=f"xf{kc}_0",
                                 name=f"xf{kc}_0")
                xfull[kc, 0] = t
                startq[(kc + 1) % 3].dma_start(t[:], xT_c[kc, :, 0:512])
            w_sb = {}
            for kc in range(CC):
                for i, nm in enumerate(("q", "k", "v")):
                    w_sb[nm, kc] = wall[:, (kc * 3 + i) * LD:
                                        (kc * 3 + i + 1) * LD]

            for w in range(1, TSEQ // 512):
                load_x_window(w)
            wout_sb = persist.tile([LD, EMB], BF16, tag="wout")

            # persistent activations (per batch)
            # QT: Q^T with head h in rows [h*64,(h+1)*64) (same layout as
            # KT); the S^T matmuls are row-tiled per head so no padding.
            QT = [persist.tile([LD, SEQ], BF16, tag=f"QT{b}",
                               name=f"QT{b}") for b in range(B)]
            KT = [persist.tile([LD, SEQ], BF16, tag=f"KT{b}", name=f"KT{b}")
                  for b in range(B)]
            outT = [persist.tile([LD, SEQ], BF16, tag=f"outT{b}",
                                 name=f"outT{b}") for b in range(B)]
            # vaug[b,kc][:, h, 0:64] = V^T chunk for head h; [:, h, 64] = 1
            # (65th stationary column accumulates softmax denominators).
            # Inner extent 66 keeps the per-head block 4B-aligned for DVE.
            vaug = {}  # (b, kc) -> [128, 2, 66] tile
            for b in range(B):
                for kc in range(NK):
                    vaug[b, kc] = persist.tile([128, 2, 66], BF16,
                                               tag=f"vaug{b}_{kc}",
                                               name=f"vaug{b}_{kc}")
                    nc.vector.memset(vaug[b, kc][:, :, 64:65], 1.0)

            def qkv_parts(b, sc, nm):
                """Emit closures for one (batch, window, tensor) projection,
                split into ~1us parts so they interleave finely with the
                attention loop (keeps ScalarE fed). Part 1 opens a misc-pool
                accumulator that part 2 closes; the phase_bc slotting
                guarantees at most one other misc allocation in between
                (bufs=2), so the open buffer is never recycled early."""
                s0 = sc * 512
                g0 = b * SEQ + s0
                cell = {}

                def p1():
                    ps = ps_misc.tile([128, 512], F32, tag="misc")
                    cell["ps"] = ps
                    for kc in range(CC // 2):
                        nc.tensor.matmul(
                            ps[:], w_sb[nm, kc], xfull[kc, g0 // 512][:],
                            start=(kc == 0), stop=False)

                def p2():
                    ps = cell["ps"]
                    for kc in range(CC // 2, CC):
                        nc.tensor.matmul(
                            ps[:], w_sb[nm, kc], xfull[kc, g0 // 512][:],
                            start=False, stop=(kc == CC - 1))
                    if nm == "q":
                        nc.vector.tensor_scalar_add(
                            QT[b][:, s0:s0 + 512], ps[:], bias_sb["q"])
                    elif nm == "k":
                        nc.vector.tensor_scalar_add(
                            KT[b][:, s0:s0 + 512], ps[:], bias_sb["k"])
                    else:
                        vt = vtp.tile([128, 512], BF16, tag="vt")
                        nc.vector.tensor_scalar_add(vt[:], ps[:],
                                                    bias_sb["v"])
                        cell["vt"] = vt

                def pt_():
                    vt = cell["vt"]
                    pst4 = ps_misc.tile([128, 4, 2, D], BF16, tag="misc")
                    for j in range(4):
                        nc.tensor.transpose(
                            pst4[:, j], vt[:, j * 128:(j + 1) * 128],
                            ident[:])
                    for j in range(4):
                        nc.vector.tensor_copy(
                            vaug[b, sc * 4 + j][:, :, 0:D], pst4[:, j])

                return [p1, p2] + ([pt_] if nm == "v" else [])

            def phase_a_units(b, scs, names=("q", "k", "v")):
                return [p for sc in scs for nm in names
                        for p in qkv_parts(b, sc, nm)]

            pending = []

            def phase_bc(b, fill_units, pre=None, flush=True):
                """Attention for batch b; fill_units and the previous
                q-chunk's projection are injected inside the kc loop so the
                static per-engine order keeps both PE and ACT fed. `pre`
                maps kc -> producer units that must be emitted before that
                kc group of q-chunk 0 (used to overlap the tail of the
                QKV projection with the start of attention)."""
                fill = list(fill_units)
                fi = 0
                pre = pre or {}

                def proj_unit(b, sc, n, eng=None, evict_eng=None):
                    rt = b * (SEQ // 128) + sc
                    ps = ps_misc.tile([128, 512], F32, tag="misc")
                    nc.tensor.matmul(
                        ps[:], outT[b][:, sc * 128:(sc + 1) * 128],
                        wout_sb[:, n * 512:(n + 1) * 512],
                        start=True, stop=True)
                    yt = youtp.tile([128, 512], BF16, tag="yt")
                    if evict_eng is nc.scalar:
                        nc.scalar.copy(yt[:], ps[:])
                    else:
                        nc.vector.tensor_copy(yt[:], ps[:])
                    if eng is None:
                        eng = nc.gpsimd if (sc + n) % 2 else nc.sync
                    eng.dma_start(
                        y.ap()[rt, :, n * 512:(n + 1) * 512], yt[:])

                def st_exp(q, kc):
                    """S^T pair + exp for (q-chunk, k-chunk); returns pt.
                    The two heads' K=64 matmuls go to row groups 0 and 64
                    (auto tile_position) and stream concurrently."""
                    q0 = q * QCH
                    st = ps_st.tile([128, 2 * QCH], F32, tag="st")
                    k0 = kc * 128
                    for h in range(HPC):
                        nc.tensor.matmul(
                            st[:, h * QCH:(h + 1) * QCH],
                            KT[b][h * D:(h + 1) * D, k0:k0 + 128],
                            QT[b][h * D:(h + 1) * D, q0:q0 + QCH],
                            start=True, stop=True)
                    pt = psb.tile([128, 2 * QCH], BF16, tag="pt")
                    nc.scalar.activation(pt[:], st[:],
                                         mybir.ActivationFunctionType.Exp,
                                         scale=SCALE)
                    return pt

                pre_pts = {}
                for q in range(NQ):
                    q0 = q * QCH
                    pvs = [ps_pv.tile([D + 1, QCH], F32, tag=f"pv{h}",
                                      name=f"pv{h}") for h in range(HPC)]
                    for kc in range(NK):
                        if q == 0:
                            for u in pre.get(kc, ()):
                                u()
                        pt = pre_pts.pop((q, kc), None)
                        if pt is None:
                            pt = st_exp(q, kc)
                        # proj/fill work first: it never waits on this
                        # iteration's exp, so PE chews it while ScalarE
                        # drains its queue.
                        if kc % 2 == 1 and pending:
                            pending.pop(0)()
                        if q > 0 and kc >= 1 and fi < len(fill):
                            fill[fi]()
                            fi += 1
                        # keep TWO S^T/exp groups in flight ahead of the PV
                        # consumer: S(kc+2) gates on exp(kc) being read out
                        # of its PSUM bank -- the same event PV(kc) waits
                        # for -- so the deeper lookahead costs PE nothing
                        # and gives ScalarE a two-deep queue.
                        for ahead in (1, 2):
                            nkc = kc + ahead
                            if nkc < NK and (q, nkc) not in pre_pts:
                                pre_pts[q, nkc] = st_exp(q, nkc)
                        for h in range(HPC):
                            nc.tensor.matmul(
                                pvs[h][:],
                                vaug[b, kc][:, h, 0:D + 1],
                                pt[:, h * QCH:(h + 1) * QCH],
                                start=(kc == 0), stop=(kc == NK - 1))
                    # pre-issue the next q-chunk's first S^T/exp groups so
                    # ScalarE stays fed across the norm+projection boundary
                    if q + 1 < NQ:
                        for kc in (0, 1):
                            pre_pts[q + 1, kc] = st_exp(q + 1, kc)
                    # normalize: out^T[d, q] / colsum -> outT (fp32r).
                    # 1/colsum on DVE straight from PSUM, partition-broadcast
                    # down the 64 rows on the (otherwise idle) GpSimd engine.
                    # Evict both pvs banks FIRST (ss + pe copies): the next
                    # q-chunk's first PV reuses these banks, so holding them
                    # through the serial recip/broadcast/mul chain would
                    # stall the PE queue at every q boundary.
                    sss, pes = [], []
                    for h in range(HPC):
                        ss = normp.tile([1, QCH], F32, tag="ss",
                                        name=f"ss{h}")
                        nc.vector.tensor_copy(ss[:], pvs[h][D:D + 1, :])
                        pe = normp.tile([D, QCH], BF16, tag="pe",
                                        name=f"pe{h}")
                        nc.vector.tensor_copy(pe[:], pvs[h][0:D, :])
                        sss.append(ss)
                        pes.append(pe)
                    for h in range(HPC):
                        rcs = normp.tile([1, QCH], F32, tag="rcs")
                        nc.vector.reciprocal_approx_fast(rcs[:], sss[h][:])
                        rb = normp.tile([D, QCH], F32, tag="rb")
                        nc.gpsimd.partition_broadcast(rb[:], rcs[:])
                        nc.vector.tensor_mul(
                            outT[b][h * D:(h + 1) * D, q0:q0 + QCH],
                            pes[h][:], rb[:])
                    pending.extend(
                        (lambda b=b, sc=sc, n=n, eng=None, evict_eng=None:
                         proj_unit(b, sc, n, eng, evict_eng))
                        for sc in range(4 * q, 4 * q + 4)
                        for n in range(EMB // 512))
                while fi < len(fill):
                    fill[fi]()
                    fi += 1
                if flush:
                    # ScalarE is idle once the last exp retires: give it
                    # half the final PSUM evictions (DVE is the tail's
                    # serial chain otherwise)
                    engs = [nc.scalar, nc.sync, nc.gpsimd]
                    evicts = [nc.vector, nc.scalar]
                    for j, p in enumerate(pending):
                        p(eng=engs[j % 3], evict_eng=evicts[j % 2])
                    del pending[:]

            for u in phase_a_units(0, [0], names=("k", "q", "v")):
                u()
            nc.sync.dma_start(wout_sb[:], wout.ap())
            # q=0 pre schedule: each part lands 1-2 iterations before its
            # first consumer; collision iterations keep the close-then-open
            # misc-buffer order (see qkv_parts).
            pre0 = {}
            for s in (1, 2, 3):
                Kp = qkv_parts(0, s, "k")
                Vp = qkv_parts(0, s, "v")
                Qp = qkv_parts(0, s, "q")
                pre0.setdefault(4 * s - 3, []).append(Kp[0])
                pre0.setdefault(4 * s - 2, []).extend([Kp[1], Vp[0]])
                pre0.setdefault(4 * s - 1, []).append(Vp[1])
                pre0.setdefault(4 * s, []).append(Vp[2])
                pre0.setdefault(4 * s + 1, []).append(Qp[0])
                pre0.setdefault(4 * s + 2, []).append(Qp[1])
            phase_bc(0, phase_a_units(1, range(4)), pre=pre0, flush=False)
            phase_bc(1, [])
            if dbg:
                nc.sync.dma_start(d_kt.ap(), KT[0][:])
                nc.sync.dma_start(d_qt.ap(), QT[0][:])
                for kc in range(NK):
                    nc.sync.dma_start(d_va.ap()[kc], vaug[0, kc][:])
                nc.sync.dma_start(d_ot.ap(), outT[0][:])

    nc.compile()
    return nc


_NC = None


def _get_nc():
    global _NC
    if _NC is None:
        _NC = _build()
    return _NC


def kernel(x, W_qkv, b_qkv, W_out, b_out):
    x = np.asarray(x, dtype=np.float32)
    W_qkv = np.asarray(W_qkv, dtype=np.float32)
    b_qkv = np.asarray(b_qkv, dtype=np.float32)
    W_out = np.asarray(W_out, dtype=np.float32)
    b_out = np.asarray(b_out, dtype=np.float32)

    nc = _get_nc()

    xT = np.ascontiguousarray(
        x.reshape(TSEQ, EMB).T.astype(NPBF16)).reshape(CC, 128, TSEQ)
    Wr = W_qkv.reshape(EMB, 3, HEADS, D)
    br = b_qkv.reshape(3, HEADS, D)

    in_maps = []
    for c in range(NCORES):
        h0, h1 = HPC * c, HPC * (c + 1)
        in_maps.append({
            "xT": xT,
            "wqkv": np.ascontiguousarray(
                np.stack([Wr[:, i, h0:h1].reshape(CC, 128, LD)
                          for i in range(3)], axis=1)
                .transpose(2, 0, 1, 3).reshape(128, CC * 3 * LD)
            ).astype(NPBF16),
            "bqkv": np.ascontiguousarray(
                np.stack([br[i, h0:h1].reshape(LD) for i in range(3)],
                         axis=1)),
            "wout": W_out[LD * c:LD * (c + 1)].astype(NPBF16),
        })

    res = bass_utils.run_bass_kernel_spmd(
        nc, in_maps, core_ids=list(range(NCORES)), trace=False)

    acc = np.zeros((TSEQ // 128, 128, EMB), dtype=np.float64)
    for c in range(NCORES):
        acc += res.results[c]["y"].astype(np.float64)
    out = (acc.reshape(TSEQ, EMB) + b_out).astype(np.float32)
    return out.reshape(B, SEQ, EMB)



# revision 88
# speedup vs baseline: 1.0072x; 1.0072x over previous
"""Multi-head attention (B=2, N=2048, C=1024, H=16) on 8 trn2 NeuronCores.

Tensor-parallel over heads: core c computes heads {2c, 2c+1} for both batch
elements and emits a partial output y_c = attn_out_c @ W_out[local rows]
(bf16 partials); the host sums the 8 partials and adds b_out.

Per-core pipeline (single TileContext, fully unrolled):
  - x^T loaded once into SBUF (bf16, host pre-transposed so every DMA is a
    contiguous [128, 512] block); weight chunks and window-0 x interleave
    on opposite DMA queues so the first QKV chain starts ASAP.
  - QKV^T projection with stacked per-head weights ([128, 128] stationary).
  - S^T = K @ Q^T as a pair of K=64 matmuls row-tiled onto disjoint row
    groups of the PE array (head h in rows h*64.. via tile_position
    auto-derived from base partitions): the pair streams concurrently, so
    it costs ~one N=512 matmul (~217ns) instead of two.
  - P^T = exp(S^T / 32) on ScalarE straight from PSUM ([128, 1024] ops,
    ~1.11us each; 128 of them = ~142us, the critical engine chain).
  - PV via ones-augmented V (65th stationary column accumulates softmax
    denominators for free). V transposed on the PE (128x128 tiles).
  - Normalization: evict ss+pe FIRST (frees the pvs PSUM banks the next
    q-chunk's PV needs), then reciprocal_approx_fast + GpSimd
    partition_broadcast + DVE multiply -> out^T (bf16).
  - Output projection all-bf16 (fp32 moving operands stream at half rate,
    so bf16 wout/outT halve both the matmul and its LDWEIGHTS).

Scheduling (emission order IS per-engine program order for Tile):
  - Each iteration emits S^T/exp for kc+1 and kc+2 before the PV pair of
    kc: the S stationaries load while the previous pair streams, and
    ScalarE holds a two-deep exp queue (it gates the steady state).
  - QKV work for batch 1 and the deferred projection stores are split
    into ~1us parts injected one-per-iteration into batch-0's attention
    loop; the misc PSUM pool (2 bufs) requires at most one other misc
    allocation between a part that opens a chain accumulator and the part
    that closes it -- the slotting here guarantees that.
  - The deferred-projection queue is shared across the two batches so
    batch-0's tail projections drain inside batch-1's loop.
Never emit a consumer before its producer: reads of not-yet-written SBUF
regions silently bind to stale contents.

Measured: ~220us on-device in the fast clock state (~264us when the part
lands in the throttled P0 state; both states observed for identical
binaries). PE union ~176us, ScalarE ~150us. absmax error ~5.2e-3 of the
output scale vs the fp32 reference (bf16 operands + fp8-free attention:
fp8 Q/K was tried and REJECTED -- softmax washout does not protect the
max-error metric on concentrated-attention queries, rel err hit 2.3e-2).
"""
import os
import sys

sys.path.insert(0, "/opt/trn_rl_repo")

import ml_dtypes
import numpy as np

import concourse.bacc as bacc
import concourse.mybir as mybir
import concourse.tile as tile
from concourse import bass_utils
from concourse.masks import make_identity

F32 = mybir.dt.float32
F32R = mybir.dt.float32r
BF16 = mybir.dt.bfloat16
F8E4 = mybir.dt.float8e4
NPBF16 = ml_dtypes.bfloat16
NPF8E4 = ml_dtypes.float8_e4m3
DR = mybir.MatmulPerfMode.DoubleRow

EMB = 1024
HEADS = 16
B = 2
SEQ = 2048
D = 64
NCORES = 8
HPC = HEADS // NCORES          # heads per core = 2
LD = HPC * D                   # local head dim = 128
TSEQ = B * SEQ                 # 4096
CC = EMB // 128                # contraction chunks = 8
SCALE = float(EMB) ** -0.5     # 1/32

QCH = 512                      # q chunk (free dim of S^T matmuls)
NQ = SEQ // QCH                # 4 q-chunks per batch
NK = SEQ // 128                # 16 k-chunks per batch


def _round_fp32r(x: np.ndarray) -> np.ndarray:
    bits = np.ascontiguousarray(x, dtype=np.float32).view(np.uint32)
    out = ((bits.astype(np.uint64) + 0x800) & 0xFFFFF000).astype(np.uint32)
    return out.view(np.float32)


def _build():
    nc = bacc.Bacc("TRN2", target_bir_lowering=False, debug=False,
                   num_devices=NCORES)

    xT = nc.dram_tensor("xT", [CC, 128, TSEQ], BF16, kind="ExternalInput")
    wqkv = nc.dram_tensor("wqkv", [128, CC * 3 * LD], BF16,
                          kind="ExternalInput")
    bqkv = nc.dram_tensor("bqkv", [LD, 3], F32, kind="ExternalInput")
    wout = nc.dram_tensor("wout", [LD, EMB], BF16, kind="ExternalInput")
    y = nc.dram_tensor("y", [TSEQ // 128, 128, EMB], BF16,
                       kind="ExternalOutput")
    dbg = os.environ.get("KDBG") == "1"
    if dbg:
        d_kt = nc.dram_tensor("d_kt", [LD, SEQ], BF16, kind="ExternalOutput")
        d_qt = nc.dram_tensor("d_qt", [LD, SEQ], BF16, kind="ExternalOutput")
        d_va = nc.dram_tensor("d_va", [NK, 128, 2, 66], BF16,
                              kind="ExternalOutput")
        d_ot = nc.dram_tensor("d_ot", [LD, SEQ], BF16,
                              kind="ExternalOutput")

    xT_c = xT.ap()
    wqkv_c = wqkv.ap()

    with tile.TileContext(nc) as tc:
        with (
            tc.tile_pool(name="persist", bufs=1) as persist,
            tc.tile_pool(name="xt", bufs=2) as xtp,
            tc.tile_pool(name="vt", bufs=2) as vtp,
            tc.tile_pool(name="psb", bufs=6) as psb,
            tc.tile_pool(name="norm", bufs=3) as normp,
            tc.tile_pool(name="yout", bufs=10) as youtp,
            tc.tile_pool(name="ps_st", bufs=2, space="PSUM") as ps_st,
            tc.tile_pool(name="ps_pv", bufs=1, space="PSUM") as ps_pv,
            tc.tile_pool(name="ps_misc", bufs=2, space="PSUM") as ps_misc,
        ):
            # ---- constants / weights (wall+bias first so QKV can start) ----
            ident = persist.tile([128, 128], BF16, tag="ident")
            make_identity(nc, ident[:])
            bqkv_sb = persist.tile([LD, 3], F32, tag="bqkv")
            nc.gpsimd.dma_start(bqkv_sb[:], bqkv.ap())
            bias_sb = {nm: bqkv_sb[:, i:i + 1]
                       for i, nm in enumerate(("q", "k", "v"))}
            wall = persist.tile([128, CC * 3 * LD], BF16, tag="wall")
            wchunk = 3 * LD

            xfull = {}

            def load_x_window(w):
                for kc in range(CC):
                    t = persist.tile([128, 512], BF16, tag=f"xf{kc}_{w}",
                                     name=f"xf{kc}_{w}")
                    eng = nc.gpsimd if kc % 2 else nc.sync
                    eng.dma_start(t[:], xT_c[kc, :, w * 512:(w + 1) * 512])
                    xfull[kc, w] = t

            # interleave weight chunks and window-0 x tiles on opposite
            # queues so the first QKV chain's full input set lands ASAP
            for kc in range(CC):
                weng = nc.gpsimd if kc % 2 else nc.sync
                weng.dma_start(wall[:, kc * wchunk:(kc + 1) * wchunk],
                               wqkv_c[:, kc * wchunk:(kc + 1) * wchunk])
                t = persist.tile([128, 512], BF16, tag=f"xf{kc}_0",
                                 name=f"xf{kc}_0")
                xeng = nc.sync if kc % 2 else nc.gpsimd
                xeng.dma_start(t[:], xT_c[kc, :, 0:512])
                xfull[kc, 0] = t
            w_sb = {}
            for kc in range(CC):
                for i, nm in enumerate(("q", "k", "v")):
                    w_sb[nm, kc] = wall[:, (kc * 3 + i) * LD:
                                        (kc * 3 + i + 1) * LD]

            for w in range(1, TSEQ // 512):
                load_x_window(w)
            wout_sb = persist.tile([LD, EMB], BF16, tag="wout")

            # persistent activations (per batch)
            # QT: Q^T with head h in rows [h*64,(h+1)*64) (same layout as
            # KT); the S^T matmuls are row-tiled per head so no padding.
            QT = [persist.tile([LD, SEQ], BF16, tag=f"QT{b}",
                               name=f"QT{b}") for b in range(B)]
            KT = [persist.tile([LD, SEQ], BF16, tag=f"KT{b}", name=f"KT{b}")
                  for b in range(B)]
            outT = [persist.tile([LD, SEQ], BF16, tag=f"outT{b}",
                                 name=f"outT{b}") for b in range(B)]
            # vaug[b,kc][:, h, 0:64] = V^T chunk for head h; [:, h, 64] = 1
            # (65th stationary column accumulates softmax denominators).
            # Inner extent 66 keeps the per-head block 4B-aligned for DVE.
            vaug = {}  # (b, kc) -> [128, 2, 66] tile
            for b in range(B):
                for kc in range(NK):
                    vaug[b, kc] = persist.tile([128, 2, 66], BF16,
                                               tag=f"vaug{b}_{kc}",
                                               name=f"vaug{b}_{kc}")
                    nc.vector.memset(vaug[b, kc][:, :, 64:65], 1.0)

            def qkv_parts(b, sc, nm):
                """Emit closures for one (batch, window, tensor) projection,
                split into ~1us parts so they interleave finely with the
                attention loop (keeps ScalarE fed). Part 1 opens a misc-pool
                accumulator that part 2 closes; the phase_bc slotting
                guarantees at most one other misc allocation in between
                (bufs=2), so the open buffer is never recycled early."""
                s0 = sc * 512
                g0 = b * SEQ + s0
                cell = {}

                def p1():
                    ps = ps_misc.tile([128, 512], F32, tag="misc")
                    cell["ps"] = ps
                    for kc in range(CC // 2):
                        nc.tensor.matmul(
                            ps[:], w_sb[nm, kc], xfull[kc, g0 // 512][:],
                            start=(kc == 0), stop=False)

                def p2():
                    ps = cell["ps"]
                    for kc in range(CC // 2, CC):
                        nc.tensor.matmul(
                            ps[:], w_sb[nm, kc], xfull[kc, g0 // 512][:],
                            start=False, stop=(kc == CC - 1))
                    if nm == "q":
                        nc.vector.tensor_scalar_add(
                            QT[b][:, s0:s0 + 512], ps[:], bias_sb["q"])
                    elif nm == "k":
                        nc.vector.tensor_scalar_add(
                            KT[b][:, s0:s0 + 512], ps[:], bias_sb["k"])
                    else:
                        vt = vtp.tile([128, 512], BF16, tag="vt")
                        nc.vector.tensor_scalar_add(vt[:], ps[:],
                                                    bias_sb["v"])
                        cell["vt"] = vt

                def pt_():
                    vt = cell["vt"]
                    pst4 = ps_misc.tile([128, 4, 2, D], BF16, tag="misc")
                    for j in range(4):
                        nc.tensor.transpose(
                            pst4[:, j], vt[:, j * 128:(j + 1) * 128],
                            ident[:])
                    for j in range(4):
                        nc.vector.tensor_copy(
                            vaug[b, sc * 4 + j][:, :, 0:D], pst4[:, j])

                return [p1, p2] + ([pt_] if nm == "v" else [])

            def phase_a_units(b, scs, names=("q", "k", "v")):
                return [p for sc in scs for nm in names
                        for p in qkv_parts(b, sc, nm)]

            pending = []

            def phase_bc(b, fill_units, pre=None, flush=True):
                """Attention for batch b; fill_units and the previous
                q-chunk's projection are injected inside the kc loop so the
                static per-engine order keeps both PE and ACT fed. `pre`
                maps kc -> producer units that must be emitted before that
                kc group of q-chunk 0 (used to overlap the tail of the
                QKV projection with the start of attention)."""
                fill = list(fill_units)
                fi = 0
                pre = pre or {}

                def proj_unit(b, sc, n, eng=None, evict_eng=None):
                    rt = b * (SEQ // 128) + sc
                    ps = ps_misc.tile([128, 512], F32, tag="misc")
                    nc.tensor.matmul(
                        ps[:], outT[b][:, sc * 128:(sc + 1) * 128],
                        wout_sb[:, n * 512:(n + 1) * 512],
                        start=True, stop=True)
                    yt = youtp.tile([128, 512], BF16, tag="yt")
                    if evict_eng is nc.scalar:
                        nc.scalar.copy(yt[:], ps[:])
                    else:
                        nc.vector.tensor_copy(yt[:], ps[:])
                    if eng is None:
                        eng = nc.gpsimd if (sc + n) % 2 else nc.sync
                    eng.dma_start(
                        y.ap()[rt, :, n * 512:(n + 1) * 512], yt[:])

                def st_exp(q, kc):
                    """S^T pair + exp for (q-chunk, k-chunk); returns pt.
                    The two heads' K=64 matmuls go to row groups 0 and 64
                    (auto tile_position) and stream concurrently."""
                    q0 = q * QCH
                    st = ps_st.tile([128, 2 * QCH], F32, tag="st")
                    k0 = kc * 128
                    for h in range(HPC):
                        nc.tensor.matmul(
                            st[:, h * QCH:(h + 1) * QCH],
                            KT[b][h * D:(h + 1) * D, k0:k0 + 128],
                            QT[b][h * D:(h + 1) * D, q0:q0 + QCH],
                            start=True, stop=True)
                    pt = psb.tile([128, 2 * QCH], BF16, tag="pt")
                    nc.scalar.activation(pt[:], st[:],
                                         mybir.ActivationFunctionType.Exp,
                                         scale=SCALE)
                    return pt

                pre_pts = {}
                for q in range(NQ):
                    q0 = q * QCH
                    pvs = [ps_pv.tile([D + 1, QCH], F32, tag=f"pv{h}",
                                      name=f"pv{h}") for h in range(HPC)]
                    for kc in range(NK):
                        if q == 0:
                            for u in pre.get(kc, ()):
                                u()
                        pt = pre_pts.pop((q, kc), None)
                        if pt is None:
                            pt = st_exp(q, kc)
                        # proj/fill work first: it never waits on this
                        # iteration's exp, so PE chews it while ScalarE
                        # drains its queue.
                        if kc % 2 == 1 and pending:
                            pending.pop(0)()
                        if q > 0 and kc >= 1 and fi < len(fill):
                            fill[fi]()
                            fi += 1
                        # keep TWO S^T/exp groups in flight ahead of the PV
                        # consumer: S(kc+2) gates on exp(kc) being read out
                        # of its PSUM bank -- the same event PV(kc) waits
                        # for -- so the deeper lookahead costs PE nothing
                        # and gives ScalarE a two-deep queue.
                        for ahead in (1, 2):
                            nkc = kc + ahead
                            if nkc < NK and (q, nkc) not in pre_pts:
                                pre_pts[q, nkc] = st_exp(q, nkc)
                        for h in range(HPC):
                            nc.tensor.matmul(
                                pvs[h][:],
                                vaug[b, kc][:, h, 0:D + 1],
                                pt[:, h * QCH:(h + 1) * QCH],
                                start=(kc == 0), stop=(kc == NK - 1))
                    # pre-issue the next q-chunk's first S^T/exp groups so
                    # ScalarE stays fed across the norm+projection boundary
                    if q + 1 < NQ:
                        for kc in (0, 1):
                            pre_pts[q + 1, kc] = st_exp(q + 1, kc)
                    # normalize: out^T[d, q] / colsum -> outT (fp32r).
                    # 1/colsum on DVE straight from PSUM, partition-broadcast
                    # down the 64 rows on the (otherwise idle) GpSimd engine.
                    # Evict both pvs banks FIRST (ss + pe copies): the next
                    # q-chunk's first PV reuses these banks, so holding them
                    # through the serial recip/broadcast/mul chain would
                    # stall the PE queue at every q boundary.
                    sss, pes = [], []
                    for h in range(HPC):
                        ss = normp.tile([1, QCH], F32, tag="ss",
                                        name=f"ss{h}")
                        nc.vector.tensor_copy(ss[:], pvs[h][D:D + 1, :])
                        pe = normp.tile([D, QCH], BF16, tag="pe",
                                        name=f"pe{h}")
                        nc.vector.tensor_copy(pe[:], pvs[h][0:D, :])
                        sss.append(ss)
                        pes.append(pe)
                    for h in range(HPC):
                        rcs = normp.tile([1, QCH], F32, tag="rcs")
                        nc.vector.reciprocal_approx_fast(rcs[:], sss[h][:])
                        rb = normp.tile([D, QCH], F32, tag="rb")
                        nc.gpsimd.partition_broadcast(rb[:], rcs[:])
                        nc.vector.tensor_mul(
                            outT[b][h * D:(h + 1) * D, q0:q0 + QCH],
                            pes[h][:], rb[:])
                    pending.extend(
                        (lambda b=b, sc=sc, n=n, eng=None, evict_eng=None:
                         proj_unit(b, sc, n, eng, evict_eng))
                        for sc in range(4 * q, 4 * q + 4)
                        for n in range(EMB // 512))
                while fi < len(fill):
                    fill[fi]()
                    fi += 1
                if flush:
                    engs = [nc.scalar, nc.sync, nc.gpsimd]
                    for j, p in enumerate(pending):
                        p(eng=engs[j % 3])
                    del pending[:]

            for u in phase_a_units(0, [0], names=("k", "q", "v")):
                u()
            nc.sync.dma_start(wout_sb[:], wout.ap())
            # q=0 pre schedule: each part lands 1-2 iterations before its
            # first consumer; collision iterations keep the close-then-open
            # misc-buffer order (see qkv_parts).
            pre0 = {}
            for s in (1, 2, 3):
                Kp = qkv_parts(0, s, "k")
                Vp = qkv_parts(0, s, "v")
                Qp = qkv_parts(0, s, "q")
                pre0.setdefault(4 * s - 3, []).append(Kp[0])
                pre0.setdefault(4 * s - 2, []).extend([Kp[1], Vp[0]])
                pre0.setdefault(4 * s - 1, []).append(Vp[1])
                pre0.setdefault(4 * s, []).append(Vp[2])
                pre0.setdefault(4 * s + 1, []).append(Qp[0])
                pre0.setdefault(4 * s + 2, []).append(Qp[1])
            phase_bc(0, phase_a_units(1, range(4)), pre=pre0, flush=False)
            phase_bc(1, [])
            if dbg:
                nc.sync.dma_start(d_kt.ap(), KT[0][:])
                nc.sync.dma_start(d_qt.ap(), QT[0][:])
                for kc in range(NK):
                    nc.sync.dma_start(d_va.ap()[kc], vaug[0, kc][:])
                nc.sync.dma_start(d_ot.ap(), outT[0][:])

    nc.compile()
    return nc


_NC = None


def _get_nc():
    global _NC
    if _NC is None:
        _NC = _build()
    return _NC


def kernel(x, W_qkv, b_qkv, W_out, b_out):
    x = np.asarray(x, dtype=np.float32)
    W_qkv = np.asarray(W_qkv, dtype=np.float32)
    b_qkv = np.asarray(b_qkv, dtype=np.float32)
    W_out = np.asarray(W_out, dtype=np.float32)
    b_out = np.asarray(b_out, dtype=np.float32)

    nc = _get_nc()

    xT = np.ascontiguousarray(
        x.reshape(TSEQ, EMB).T.astype(NPBF16)).reshape(CC, 128, TSEQ)
    Wr = W_qkv.reshape(EMB, 3, HEADS, D)
    br = b_qkv.reshape(3, HEADS, D)

    in_maps = []
    for c in range(NCORES):
        h0, h1 = HPC * c, HPC * (c + 1)
        in_maps.append({
            "xT": xT,
            "wqkv": np.ascontiguousarray(
                np.stack([Wr[:, i, h0:h1].reshape(CC, 128, LD)
                          for i in range(3)], axis=1)
                .transpose(2, 0, 1, 3).reshape(128, CC * 3 * LD)
            ).astype(NPBF16),
            "bqkv": np.ascontiguousarray(
                np.stack([br[i, h0:h1].reshape(LD) for i in range(3)],
                         axis=1)),
            "wout": W_out[LD * c:LD * (c + 1)].astype(NPBF16),
        })

    res = bass_utils.run_bass_kernel_spmd(
        nc, in_maps, core_ids=list(range(NCORES)), trace=False)

    acc = np.zeros((TSEQ // 128, 128, EMB), dtype=np.float64)
    for c in range(NCORES):
        acc += res.results[c]["y"].astype(np.float64)
    out = (acc.reshape(TSEQ, EMB) + b_out).astype(np.float32)
    return out.reshape(B, SEQ, EMB)



# revision 89
# speedup vs baseline: 1.0075x; 1.0003x over previous
"""Multi-head attention (B=2, N=2048, C=1024, H=16) on 8 trn2 NeuronCores.

Tensor-parallel over heads: core c computes heads {2c, 2c+1} for both batch
elements and emits a partial output y_c = attn_out_c @ W_out[local rows]
(bf16 partials); the host sums the 8 partials and adds b_out.

Per-core pipeline (single TileContext, fully unrolled):
  - x^T loaded once into SBUF (bf16, host pre-transposed so every DMA is a
    contiguous [128, 512] block); weight chunks and window-0 x interleave
    on opposite DMA queues so the first QKV chain starts ASAP.
  - QKV^T projection with stacked per-head weights ([128, 128] stationary).
  - S^T = K @ Q^T as a pair of K=64 matmuls row-tiled onto disjoint row
    groups of the PE array (head h in rows h*64.. via tile_position
    auto-derived from base partitions): the pair streams concurrently, so
    it costs ~one N=512 matmul (~217ns) instead of two.
  - P^T = exp(S^T / 32) on ScalarE straight from PSUM ([128, 1024] ops,
    ~1.11us each; 128 of them = ~142us, the critical engine chain).
  - PV via ones-augmented V (65th stationary column accumulates softmax
    denominators for free). V transposed on the PE (128x128 tiles).
  - Normalization: evict ss+pe FIRST (frees the pvs PSUM banks the next
    q-chunk's PV needs), then reciprocal_approx_fast + GpSimd
    partition_broadcast + DVE multiply -> out^T (bf16).
  - Output projection all-bf16 (fp32 moving operands stream at half rate,
    so bf16 wout/outT halve both the matmul and its LDWEIGHTS).

Scheduling (emission order IS per-engine program order for Tile):
  - Each iteration emits S^T/exp for kc+1 and kc+2 before the PV pair of
    kc: the S stationaries load while the previous pair streams, and
    ScalarE holds a two-deep exp queue (it gates the steady state).
  - QKV work for batch 1 and the deferred projection stores are split
    into ~1us parts injected one-per-iteration into batch-0's attention
    loop; the misc PSUM pool (2 bufs) requires at most one other misc
    allocation between a part that opens a chain accumulator and the part
    that closes it -- the slotting here guarantees that.
  - The deferred-projection queue is shared across the two batches so
    batch-0's tail projections drain inside batch-1's loop.
Never emit a consumer before its producer: reads of not-yet-written SBUF
regions silently bind to stale contents.

Measured: ~220us on-device in the fast clock state (~264us when the part
lands in the throttled P0 state; both states observed for identical
binaries). PE union ~176us, ScalarE ~150us. absmax error ~5.2e-3 of the
output scale vs the fp32 reference (bf16 operands + fp8-free attention:
fp8 Q/K was tried and REJECTED -- softmax washout does not protect the
max-error metric on concentrated-attention queries, rel err hit 2.3e-2).
"""
import os
import sys

sys.path.insert(0, "/opt/trn_rl_repo")

import ml_dtypes
import numpy as np

import concourse.bacc as bacc
import concourse.mybir as mybir
import concourse.tile as tile
from concourse import bass_utils
from concourse.masks import make_identity

F32 = mybir.dt.float32
F32R = mybir.dt.float32r
BF16 = mybir.dt.bfloat16
F8E4 = mybir.dt.float8e4
NPBF16 = ml_dtypes.bfloat16
NPF8E4 = ml_dtypes.float8_e4m3
DR = mybir.MatmulPerfMode.DoubleRow

EMB = 1024
HEADS = 16
B = 2
SEQ = 2048
D = 64
NCORES = 8
HPC = HEADS // NCORES          # heads per core = 2
LD = HPC * D                   # local head dim = 128
TSEQ = B * SEQ                 # 4096
CC = EMB // 128                # contraction chunks = 8
SCALE = float(EMB) ** -0.5     # 1/32

QCH = 512                      # q chunk (free dim of S^T matmuls)
NQ = SEQ // QCH                # 4 q-chunks per batch
NK = SEQ // 128                # 16 k-chunks per batch


def _round_fp32r(x: np.ndarray) -> np.ndarray:
    bits = np.ascontiguousarray(x, dtype=np.float32).view(np.uint32)
    out = ((bits.astype(np.uint64) + 0x800) & 0xFFFFF000).astype(np.uint32)
    return out.view(np.float32)


def _build():
    nc = bacc.Bacc("TRN2", target_bir_lowering=False, debug=False,
                   num_devices=NCORES)

    xT = nc.dram_tensor("xT", [CC, 128, TSEQ], BF16, kind="ExternalInput")
    wqkv = nc.dram_tensor("wqkv", [128, CC * 3 * LD], BF16,
                          kind="ExternalInput")
    bqkv = nc.dram_tensor("bqkv", [LD, 3], F32, kind="ExternalInput")
    wout = nc.dram_tensor("wout", [LD, EMB], BF16, kind="ExternalInput")
    y = nc.dram_tensor("y", [TSEQ // 128, 128, EMB], BF16,
                       kind="ExternalOutput")
    dbg = os.environ.get("KDBG") == "1"
    if dbg:
        d_kt = nc.dram_tensor("d_kt", [LD, SEQ], BF16, kind="ExternalOutput")
        d_qt = nc.dram_tensor("d_qt", [LD, SEQ], BF16, kind="ExternalOutput")
        d_va = nc.dram_tensor("d_va", [NK, 128, 2, 66], BF16,
                              kind="ExternalOutput")
        d_ot = nc.dram_tensor("d_ot", [LD, SEQ], BF16,
                              kind="ExternalOutput")

    xT_c = xT.ap()
    wqkv_c = wqkv.ap()

    with tile.TileContext(nc) as tc:
        with (
            tc.tile_pool(name="persist", bufs=1) as persist,
            tc.tile_pool(name="xt", bufs=2) as xtp,
            tc.tile_pool(name="vt", bufs=2) as vtp,
            tc.tile_pool(name="psb", bufs=8) as psb,
            tc.tile_pool(name="norm", bufs=3) as normp,
            tc.tile_pool(name="yout", bufs=10) as youtp,
            tc.tile_pool(name="ps_st", bufs=2, space="PSUM") as ps_st,
            tc.tile_pool(name="ps_pv", bufs=1, space="PSUM") as ps_pv,
            tc.tile_pool(name="ps_misc", bufs=2, space="PSUM") as ps_misc,
        ):
            # ---- constants / weights (wall+bias first so QKV can start) ----
            ident = persist.tile([128, 128], BF16, tag="ident")
            make_identity(nc, ident[:])
            bqkv_sb = persist.tile([LD, 3], F32, tag="bqkv")
            nc.gpsimd.dma_start(bqkv_sb[:], bqkv.ap())
            bias_sb = {nm: bqkv_sb[:, i:i + 1]
                       for i, nm in enumerate(("q", "k", "v"))}
            wall = persist.tile([128, CC * 3 * LD], BF16, tag="wall")
            wchunk = 3 * LD

            xfull = {}

            def load_x_window(w):
                for kc in range(CC):
                    t = persist.tile([128, 512], BF16, tag=f"xf{kc}_{w}",
                                     name=f"xf{kc}_{w}")
                    eng = nc.gpsimd if kc % 2 else nc.sync
                    eng.dma_start(t[:], xT_c[kc, :, w * 512:(w + 1) * 512])
                    xfull[kc, w] = t

            # interleave weight chunks and window-0 x tiles on opposite
            # queues so the first QKV chain's full input set lands ASAP
            for kc in range(CC):
                weng = nc.gpsimd if kc % 2 else nc.sync
                weng.dma_start(wall[:, kc * wchunk:(kc + 1) * wchunk],
                               wqkv_c[:, kc * wchunk:(kc + 1) * wchunk])
                t = persist.tile([128, 512], BF16, tag=f"xf{kc}_0",
                                 name=f"xf{kc}_0")
                xeng = nc.sync if kc % 2 else nc.gpsimd
                xeng.dma_start(t[:], xT_c[kc, :, 0:512])
                xfull[kc, 0] = t
            w_sb = {}
            for kc in range(CC):
                for i, nm in enumerate(("q", "k", "v")):
                    w_sb[nm, kc] = wall[:, (kc * 3 + i) * LD:
                                        (kc * 3 + i + 1) * LD]

            for w in range(1, TSEQ // 512):
                load_x_window(w)
            wout_sb = persist.tile([LD, EMB], BF16, tag="wout")

            # persistent activations (per batch)
            # QT: Q^T with head h in rows [h*64,(h+1)*64) (same layout as
            # KT); the S^T matmuls are row-tiled per head so no padding.
            QT = [persist.tile([LD, SEQ], BF16, tag=f"QT{b}",
                               name=f"QT{b}") for b in range(B)]
            KT = [persist.tile([LD, SEQ], BF16, tag=f"KT{b}", name=f"KT{b}")
                  for b in range(B)]
            outT = [persist.tile([LD, SEQ], BF16, tag=f"outT{b}",
                                 name=f"outT{b}") for b in range(B)]
            # vaug[b,kc][:, h, 0:64] = V^T chunk for head h; [:, h, 64] = 1
            # (65th stationary column accumulates softmax denominators).
            # Inner extent 66 keeps the per-head block 4B-aligned for DVE.
            vaug = {}  # (b, kc) -> [128, 2, 66] tile
            for b in range(B):
                for kc in range(NK):
                    vaug[b, kc] = persist.tile([128, 2, 66], BF16,
                                               tag=f"vaug{b}_{kc}",
                                               name=f"vaug{b}_{kc}")
                    nc.vector.memset(vaug[b, kc][:, :, 64:65], 1.0)

            def qkv_parts(b, sc, nm):
                """Emit closures for one (batch, window, tensor) projection,
                split into ~1us parts so they interleave finely with the
                attention loop (keeps ScalarE fed). Part 1 opens a misc-pool
                accumulator that part 2 closes; the phase_bc slotting
                guarantees at most one other misc allocation in between
                (bufs=2), so the open buffer is never recycled early."""
                s0 = sc * 512
                g0 = b * SEQ + s0
                cell = {}

                def p1():
                    ps = ps_misc.tile([128, 512], F32, tag="misc")
                    cell["ps"] = ps
                    for kc in range(CC // 2):
                        nc.tensor.matmul(
                            ps[:], w_sb[nm, kc], xfull[kc, g0 // 512][:],
                            start=(kc == 0), stop=False)

                def p2():
                    ps = cell["ps"]
                    for kc in range(CC // 2, CC):
                        nc.tensor.matmul(
                            ps[:], w_sb[nm, kc], xfull[kc, g0 // 512][:],
                            start=False, stop=(kc == CC - 1))
                    if nm == "q":
                        nc.vector.tensor_scalar_add(
                            QT[b][:, s0:s0 + 512], ps[:], bias_sb["q"])
                    elif nm == "k":
                        nc.vector.tensor_scalar_add(
                            KT[b][:, s0:s0 + 512], ps[:], bias_sb["k"])
                    else:
                        vt = vtp.tile([128, 512], BF16, tag="vt")
                        nc.vector.tensor_scalar_add(vt[:], ps[:],
                                                    bias_sb["v"])
                        cell["vt"] = vt

                def pt_():
                    vt = cell["vt"]
                    pst4 = ps_misc.tile([128, 4, 2, D], BF16, tag="misc")
                    for j in range(4):
                        nc.tensor.transpose(
                            pst4[:, j], vt[:, j * 128:(j + 1) * 128],
                            ident[:])
                    for j in range(4):
                        nc.vector.tensor_copy(
                            vaug[b, sc * 4 + j][:, :, 0:D], pst4[:, j])

                return [p1, p2] + ([pt_] if nm == "v" else [])

            def phase_a_units(b, scs, names=("q", "k", "v")):
                return [p for sc in scs for nm in names
                        for p in qkv_parts(b, sc, nm)]

            pending = []

            def phase_bc(b, fill_units, pre=None, flush=True):
                """Attention for batch b; fill_units and the previous
                q-chunk's projection are injected inside the kc loop so the
                static per-engine order keeps both PE and ACT fed. `pre`
                maps kc -> producer units that must be emitted before that
                kc group of q-chunk 0 (used to overlap the tail of the
                QKV projection with the start of attention)."""
                fill = list(fill_units)
                fi = 0
                pre = pre or {}

                def proj_unit(b, sc, n, eng=None, evict_eng=None):
                    rt = b * (SEQ // 128) + sc
                    ps = ps_misc.tile([128, 512], F32, tag="misc")
                    nc.tensor.matmul(
                        ps[:], outT[b][:, sc * 128:(sc + 1) * 128],
                        wout_sb[:, n * 512:(n + 1) * 512],
                        start=True, stop=True)
                    yt = youtp.tile([128, 512], BF16, tag="yt")
                    if evict_eng is nc.scalar:
                        nc.scalar.copy(yt[:], ps[:])
                    else:
                        nc.vector.tensor_copy(yt[:], ps[:])
                    if eng is None:
                        eng = nc.gpsimd if (sc + n) % 2 else nc.sync
                    eng.dma_start(
                        y.ap()[rt, :, n * 512:(n + 1) * 512], yt[:])

                def st_exp(q, kc):
                    """S^T pair + exp for (q-chunk, k-chunk); returns pt.
                    The two heads' K=64 matmuls go to row groups 0 and 64
                    (auto tile_position) and stream concurrently."""
                    q0 = q * QCH
                    st = ps_st.tile([128, 2 * QCH], F32, tag="st")
                    k0 = kc * 128
                    for h in range(HPC):
                        nc.tensor.matmul(
                            st[:, h * QCH:(h + 1) * QCH],
                            KT[b][h * D:(h + 1) * D, k0:k0 + 128],
                            QT[b][h * D:(h + 1) * D, q0:q0 + QCH],
                            start=True, stop=True)
                    pt = psb.tile([128, 2 * QCH], BF16, tag="pt")
                    nc.scalar.activation(pt[:], st[:],
                                         mybir.ActivationFunctionType.Exp,
                                         scale=SCALE)
                    return pt

                pre_pts = {}
                for q in range(NQ):
                    q0 = q * QCH
                    pvs = [ps_pv.tile([D + 1, QCH], F32, tag=f"pv{h}",
                                      name=f"pv{h}") for h in range(HPC)]
                    for kc in range(NK):
                        if q == 0:
                            for u in pre.get(kc, ()):
                                u()
                        pt = pre_pts.pop((q, kc), None)
                        if pt is None:
                            pt = st_exp(q, kc)
                        # proj/fill work first: it never waits on this
                        # iteration's exp, so PE chews it while ScalarE
                        # drains its queue.
                        if kc % 2 == 1 and pending:
                            pending.pop(0)()
                        if q > 0 and kc >= 1 and fi < len(fill):
                            fill[fi]()
                            fi += 1
                        # keep TWO S^T/exp groups in flight ahead of the PV
                        # consumer: S(kc+2) gates on exp(kc) being read out
                        # of its PSUM bank -- the same event PV(kc) waits
                        # for -- so the deeper lookahead costs PE nothing
                        # and gives ScalarE a two-deep queue.
                        for ahead in (1, 2):
                            nkc = kc + ahead
                            if nkc < NK and (q, nkc) not in pre_pts:
                                pre_pts[q, nkc] = st_exp(q, nkc)
                        for h in range(HPC):
                            nc.tensor.matmul(
                                pvs[h][:],
                                vaug[b, kc][:, h, 0:D + 1],
                                pt[:, h * QCH:(h + 1) * QCH],
                                start=(kc == 0), stop=(kc == NK - 1))
                    # pre-issue the next q-chunk's first S^T/exp groups so
                    # ScalarE stays fed across the norm+projection boundary
                    if q + 1 < NQ:
                        for kc in (0, 1):
                            pre_pts[q + 1, kc] = st_exp(q + 1, kc)
                    # normalize: out^T[d, q] / colsum -> outT (fp32r).
                    # 1/colsum on DVE straight from PSUM, partition-broadcast
                    # down the 64 rows on the (otherwise idle) GpSimd engine.
                    # Evict both pvs banks FIRST (ss + pe copies): the next
                    # q-chunk's first PV reuses these banks, so holding them
                    # through the serial recip/broadcast/mul chain would
                    # stall the PE queue at every q boundary.
                    sss, pes = [], []
                    for h in range(HPC):
                        ss = normp.tile([1, QCH], F32, tag="ss",
                                        name=f"ss{h}")
                        nc.vector.tensor_copy(ss[:], pvs[h][D:D + 1, :])
                        pe = normp.tile([D, QCH], BF16, tag="pe",
                                        name=f"pe{h}")
                        nc.vector.tensor_copy(pe[:], pvs[h][0:D, :])
                        sss.append(ss)
                        pes.append(pe)
                    for h in range(HPC):
                        rcs = normp.tile([1, QCH], F32, tag="rcs")
                        nc.vector.reciprocal_approx_fast(rcs[:], sss[h][:])
                        rb = normp.tile([D, QCH], F32, tag="rb")
                        nc.gpsimd.partition_broadcast(rb[:], rcs[:])
                        nc.vector.tensor_mul(
                            outT[b][h * D:(h + 1) * D, q0:q0 + QCH],
                            pes[h][:], rb[:])
                    pending.extend(
                        (lambda b=b, sc=sc, n=n, eng=None, evict_eng=None:
                         proj_unit(b, sc, n, eng, evict_eng))
                        for sc in range(4 * q, 4 * q + 4)
                        for n in range(EMB // 512))
                while fi < len(fill):
                    fill[fi]()
                    fi += 1
                if flush:
                    engs = [nc.scalar, nc.sync, nc.gpsimd]
                    for j, p in enumerate(pending):
                        p(eng=engs[j % 3])
                    del pending[:]

            for u in phase_a_units(0, [0], names=("k", "q", "v")):
                u()
            nc.sync.dma_start(wout_sb[:], wout.ap())
            # q=0 pre schedule: each part lands 1-2 iterations before its
            # first consumer; collision iterations keep the close-then-open
            # misc-buffer order (see qkv_parts).
            pre0 = {}
            for s in (1, 2, 3):
                Kp = qkv_parts(0, s, "k")
                Vp = qkv_parts(0, s, "v")
                Qp = qkv_parts(0, s, "q")
                pre0.setdefault(4 * s - 3, []).append(Kp[0])
                pre0.setdefault(4 * s - 2, []).extend([Kp[1], Vp[0]])
                pre0.setdefault(4 * s - 1, []).append(Vp[1])
                pre0.setdefault(4 * s, []).append(Vp[2])
                pre0.setdefault(4 * s + 1, []).append(Qp[0])
                pre0.setdefault(4 * s + 2, []).append(Qp[1])
            phase_bc(0, phase_a_units(1, range(4)), pre=pre0, flush=False)
            phase_bc(1, [])
            if dbg:
                nc.sync.dma_start(d_kt.ap(), KT[0][:])
                nc.sync.dma_start(d_qt.ap(), QT[0][:])
                for kc in range(NK):
                    nc.sync.dma_start(d_va.ap()[kc], vaug[0, kc][:])
                nc.sync.dma_start(d_ot.ap(), outT[0][:])

    nc.compile()
    return nc


_NC = None


def _get_nc():
    global _NC
    if _NC is None:
        _NC = _build()
    return _NC


def kernel(x, W_qkv, b_qkv, W_out, b_out):
    x = np.asarray(x, dtype=np.float32)
    W_qkv = np.asarray(W_qkv, dtype=np.float32)
    b_qkv = np.asarray(b_qkv, dtype=np.float32)
    W_out = np.asarray(W_out, dtype=np.float32)
    b_out = np.asarray(b_out, dtype=np.float32)

    nc = _get_nc()

    xT = np.ascontiguousarray(
        x.reshape(TSEQ, EMB).T.astype(NPBF16)).reshape(CC, 128, TSEQ)
    Wr = W_qkv.reshape(EMB, 3, HEADS, D)
    br = b_qkv.reshape(3, HEADS, D)

    in_maps = []
    for c in range(NCORES):
        h0, h1 = HPC * c, HPC * (c + 1)
        in_maps.append({
            "xT": xT,
            "wqkv": np.ascontiguousarray(
                np.stack([Wr[:, i, h0:h1].reshape(CC, 128, LD)
                          for i in range(3)], axis=1)
                .transpose(2, 0, 1, 3).reshape(128, CC * 3 * LD)
            ).astype(NPBF16),
            "bqkv": np.ascontiguousarray(
                np.stack([br[i, h0:h1].reshape(LD) for i in range(3)],
                         axis=1)),
            "wout": W_out[LD * c:LD * (c + 1)].astype(NPBF16),
        })

    res = bass_utils.run_bass_kernel_spmd(
        nc, in_maps, core_ids=list(range(NCORES)), trace=False)

    acc = np.zeros((TSEQ // 128, 128, EMB), dtype=np.float64)
    for c in range(NCORES):
        acc += res.results[c]["y"].astype(np.float64)
    out = (acc.reshape(TSEQ, EMB) + b_out).astype(np.float32)
    return out.reshape(B, SEQ, EMB)

